# revision 1
# baseline (speedup 1.0000x reference)
"""Self-contained Trainium2 Bass kernel for the 5-layer GraphConv GNN
(N=100000 nodes, E=3200000 edges, dims 6->20->15->10->5->2, relu, softmax).

kernel(**inputs) takes the full unsharded inputs (as from setup_inputs()),
shards edges across 8 NeuronCores by destination-node range internally,
runs the Bass program via run_bass_kernel_spmd, and returns the full
[100000, 2] float32 output.
"""

import sys
sys.path.insert(0, '/opt/trn_rl_repo')
import numpy as np
import concourse.bass as bass
import concourse.bacc as bacc
import concourse.tile as tile
from concourse import mybir

f32 = mybir.dt.float32
f16 = mybir.dt.float16
i32 = mybir.dt.int32
i16 = mybir.dt.int16

NCHUNK = 4
PAD = 64  # table row padded to 64 f32 = 256 bytes (dma_gather elem quantum)


def preprocess(edge_index, edge_weight, N, ncores):
    """Group edges by (core, dst-block, src-chunk); pad each (block, chunk)
    group to T_chunk tiles of 128 slots (dummy slots: src 0, w 0).

    Slot (b, s) with s = t*128 + p lives at array position [p, b*TPB + t]
    where TPB = NCHUNK*T_chunk tiles per block. Chunk c owns tiles
    [c*T_chunk, (c+1)*T_chunk).

    Returns per-core arrays:
      idx16 [ncores, NB, 128, NCHUNK*T_chunk*8] int16 (gather indices,
            wrapped j%16 -> partition, j//16 -> column, replicated x8)
      dstl  [ncores, 128, NB*TPB] f16
      wgt   [ncores, 128, NB*TPB] f32
    plus (T_chunk, NB, NPC, last_cnt, CH).
    """
    src = np.asarray(edge_index[0], dtype=np.int64)
    dst = np.asarray(edge_index[1], dtype=np.int64)
    w = np.asarray(edge_weight, dtype=np.float32)
    NPC = N // ncores
    NB = (NPC + 127) // 128
    last_cnt = NPC - (NB - 1) * 128
    CH = (N + NCHUNK - 1) // NCHUNK

    core = dst // NPC
    r = dst % NPC
    blk = r // 128
    loc = r % 128
    gb = core * NB + blk
    chunk = src // CH
    srcl = src - chunk * CH
    key = gb * NCHUNK + chunk
    order = np.argsort(key, kind='stable')
    srcl_s, loc_s, w_s, key_s = srcl[order], loc[order], w[order], key[order]

    ngroups = ncores * NB * NCHUNK
    counts = np.bincount(key_s, minlength=ngroups)
    T_chunk = max(1, int(np.ceil(counts.max() / 128)))
    SP = T_chunk * 128            # slots per (block, chunk) group
    S = NCHUNK * SP               # slots per block
    TPB = NCHUNK * T_chunk        # tiles per block

    nblocks = ncores * NB
    srcl_p = np.zeros((ngroups, SP), np.int16)
    loc_p = np.zeros((ngroups, SP), np.float16)
    w_p = np.zeros((ngroups, SP), np.float32)
    starts = np.concatenate([[0], np.cumsum(counts)[:-1]])
    pos = np.arange(len(key_s)) - starts[key_s]
    srcl_p[key_s, pos] = srcl_s
    loc_p[key_s, pos] = loc_s
    w_p[key_s, pos] = w_s

    # idx16: per (block, chunk) instruction, index j (= slot within group)
    # goes to [j % 16, j // 16]; replicate the 16 rows x8 to 128 partitions.
    idx_wrap = np.ascontiguousarray(
        srcl_p.reshape(ngroups, SP // 16, 16).transpose(0, 2, 1))  # [g, 16, SP/16]
    idx_wrap = np.tile(idx_wrap, (1, 8, 1))  # [g, 128, SP/16]
    idx16 = idx_wrap.reshape(ncores, NB, NCHUNK, 128, SP // 16)
    idx16 = np.ascontiguousarray(
        idx16.transpose(0, 1, 3, 2, 4).reshape(ncores, NB, 128, NCHUNK * (SP // 16)))

    def to_sbuf(a):
        # [ngroups, SP] -> [ncores, 128, NB*TPB] with [p, b*TPB+t] = slot t*128+p
        a = a.reshape(ncores, NB, NCHUNK, T_chunk, 128)
        return np.ascontiguousarray(
            a.transpose(0, 4, 1, 2, 3).reshape(ncores, 128, NB * TPB))

    return idx16, to_sbuf(loc_p), to_sbuf(w_p), T_chunk, NB, NPC, last_cnt, CH


def build_gnn(nc, N, NPC, NB, T_chunk, dims, ncores, last_cnt, CH, debug=False):
    """Emit the full 5-layer program into nc."""
    L = len(dims) - 1
    TPB = NCHUNK * T_chunk
    SP = T_chunk * 128
    dbg_d = {}
    if debug:
        for l in range(L):
            dbg_d[f"dbg_ytab{l}"] = nc.dram_tensor(
                f"dbg_ytab{l}", [N, dims[l + 1]], f32, kind="ExternalOutput")
            if l < L - 1:
                dbg_d[f"dbg_xT{l + 1}"] = nc.dram_tensor(
                    f"dbg_xT{l + 1}", [dims[l + 1], NPC], f32, kind="ExternalOutput")
            dbg_d[f"dbg_agg{l}"] = nc.dram_tensor(
                f"dbg_agg{l}", [NB * 128, dims[l + 1]], f32, kind="ExternalOutput")

    # ---- DRAM I/O ----
    idx_d = nc.dram_tensor("idx16", [NB, 128, NCHUNK * (SP // 16)], i16, kind="ExternalInput")
    dstl_d = nc.dram_tensor("dstl", [128, NB * TPB], f16, kind="ExternalInput")
    wgt_d = nc.dram_tensor("wgt", [128, NB * TPB], f32, kind="ExternalInput")
    xT0_d = nc.dram_tensor("xT0", [dims[0], NPC], f32, kind="ExternalInput")
    x64_d = nc.dram_tensor("x64", [N, PAD], f32, kind="ExternalInput")
    iota_d = nc.dram_tensor("iota", [128, 128], f16, kind="ExternalInput")
    ident_d = nc.dram_tensor("ident", [128, 128], f32, kind="ExternalInput")
    wrel_d = [nc.dram_tensor(f"wrel{l}", [dims[l], dims[l + 1]], f32, kind="ExternalInput") for l in range(L)]
    wroot_d = [nc.dram_tensor(f"wroot{l}", [dims[l], dims[l + 1]], f32, kind="ExternalInput") for l in range(L)]
    brel_d = [nc.dram_tensor(f"brel{l}", [dims[l + 1], 1], f32, kind="ExternalInput") for l in range(L)]
    out_d = nc.dram_tensor("out", [NPC, dims[L]], f32, kind="ExternalOutput")

    groups = [list(range(ncores))]

    with tile.TileContext(nc) as tc:
        with (
            tc.tile_pool(name="const", bufs=1) as cpool,
            tc.tile_pool(name="edge", bufs=1) as epool,
            tc.tile_pool(name="xts", bufs=1) as xpool,
            tc.tile_pool(name="idxp", bufs=3) as ipool,
            tc.tile_pool(name="gath", bufs=3) as gpool,
            tc.tile_pool(name="msg", bufs=2) as mpool,
            tc.tile_pool(name="ohp", bufs=2) as opool,
            tc.tile_pool(name="small", bufs=3) as spool,
            tc.tile_pool(name="psum", bufs=1, space="PSUM") as ppool,
            tc.tile_pool(name="dram", bufs=1, space="DRAM") as dpool,
        ):
            # ---- load constants / edge data ----
            dstl_sb = epool.tile([128, NB * TPB], f16)
            nc.sync.dma_start(dstl_sb[:], dstl_d.ap()[:])
            wgt_sb = epool.tile([128, NB * TPB], f32)
            nc.sync.dma_start(wgt_sb[:], wgt_d.ap()[:])
            iota_sb = cpool.tile([128, 128], f16)
            nc.sync.dma_start(iota_sb[:], iota_d.ap()[:])
            ident_sb = cpool.tile([128, 128], f32)
            nc.sync.dma_start(ident_sb[:], ident_d.ap()[:])
            wrel_sb, wroot_sb, brel_sb = [], [], []
            for l in range(L):
                t1 = cpool.tile([dims[l], dims[l + 1]], f32, name=f"wrel_sb{l}")
                nc.sync.dma_start(t1[:], wrel_d[l].ap()[:])
                wrel_sb.append(t1)
                t2 = cpool.tile([dims[l], dims[l + 1]], f32, name=f"wroot_sb{l}")
                nc.sync.dma_start(t2[:], wroot_d[l].ap()[:])
                wroot_sb.append(t2)
                t3 = cpool.tile([dims[l + 1], 1], f32, name=f"brel_sb{l}")
                nc.sync.dma_start(t3[:], brel_d[l].ap()[:])
                brel_sb.append(t3)

            xT_cur = xpool.tile([dims[0], NPC], f32, name="xT_l0", tag="xT", bufs=2)
            nc.sync.dma_start(xT_cur[:], xT0_d.ap()[:])

            for l in range(L):
                c = dims[l + 1]
                din = dims[l]
                if l == 0:
                    ytab64 = x64_d.ap()
                else:
                    # ---- y = x @ W_rel locally, AllGather, restride to 256B rows
                    ybounce = dpool.tile([NPC, c], f32, name=f"ybounce{l}")
                    for b in range(NB):
                        cnt = 128 if b < NB - 1 else last_cnt
                        yps = ppool.tile([128, c], f32, name=f"yps{l}_{b}", tag="y", bufs=2)
                        nc.tensor.matmul(
                            out=yps[:cnt, :],
                            lhsT=xT_cur[:, b * 128:b * 128 + cnt],
                            rhs=wrel_sb[l][:],
                            start=True, stop=True,
                        )
                        ysb = spool.tile([128, c], f32, name=f"ysb{l}_{b}", tag="ysb")
                        nc.vector.tensor_copy(ysb[:cnt, :], yps[:cnt, :])
                        nc.sync.dma_start(ybounce[b * 128:b * 128 + cnt, :], ysb[:cnt, :])
                    ytabc = dpool.tile([N, c], f32, name=f"ytabc{l}")
                    nc.gpsimd.collective_compute(
                        "AllGather",
                        mybir.AluOpType.bypass,
                        replica_groups=groups,
                        ins=[ybounce[:].opt()],
                        outs=[ytabc[:].opt()],
                    )
                    ytab64_t = dpool.tile([N, PAD], f32, name=f"ytab64_{l}")
                    for ch in range(NCHUNK):
                        r0, r1 = ch * CH, min((ch + 1) * CH, N)
                        nc.sync.dma_start(ytab64_t[r0:r1, :c], ytabc[r0:r1, :])
                    ytab64 = ytab64_t

                # ---- main per-block loop ----
                if l < L - 1:
                    xT_next = xpool.tile([c, NPC], f32, name=f"xT_l{l + 1}", tag="xT", bufs=2)
                else:
                    xT_next = None
                for b in range(NB):
                    cnt = 128 if b < NB - 1 else last_cnt
                    es = slice(b * TPB, (b + 1) * TPB)
                    idx_sb = ipool.tile([128, NCHUNK * (SP // 16)], i16, name=f"idx{l}_{b}", tag="idx")
                    nc.sync.dma_start(idx_sb[:], idx_d.ap()[b])
                    gth = gpool.tile([128, TPB * PAD], f32, name=f"gth{l}_{b}", tag="gth")
                    for ch in range(NCHUNK):
                        r0 = ch * CH
                        r1 = min(r0 + CH, N)
                        t0 = 0
                        while t0 < T_chunk:
                            tn = min(8, T_chunk - t0)  # dma_gather max 1024 idxs/instr
                            nc.gpsimd.dma_gather(
                                out_ap=gth[:, (ch * T_chunk + t0) * PAD:(ch * T_chunk + t0 + tn) * PAD]
                                    .rearrange("p (t e) -> p t e", e=PAD),
                                in_ap=ytab64[r0:r1, :],
                                idxs_ap=idx_sb[:, ch * (SP // 16) + t0 * 8:ch * (SP // 16) + (t0 + tn) * 8],
                                num_idxs=tn * 128,
                                num_idxs_reg=tn * 128,
                                elem_size=PAD,
                            )
                            t0 += tn
                    msg = mpool.tile([128, TPB * c], f16, name=f"msg{l}_{b}", tag="msg")
                    nc.vector.tensor_tensor(
                        out=msg[:].rearrange("p (t c) -> p t c", c=c),
                        in0=gth[:].rearrange("p (t e) -> p t e", e=PAD)[:, :, :c],
                        in1=wgt_sb[:, es].to_broadcast([128, TPB, c]),
                        op=mybir.AluOpType.mult,
                    )
                    oh = opool.tile([128, TPB * 128], f16, name=f"oh{l}_{b}", tag="oh")
                    nc.vector.tensor_tensor(
                        out=oh[:].rearrange("p (t n) -> p t n", n=128),
                        in0=dstl_sb[:, es].to_broadcast([128, TPB, 128]),
                        in1=iota_sb[:, None, :].to_broadcast([128, TPB, 128]),
                        op=mybir.AluOpType.is_equal,
                    )
                    aggps = ppool.tile([128, c], f32, name=f"aggps{l}_{b}", tag="agg", bufs=2)
                    for t in range(TPB):
                        nc.tensor.matmul(
                            out=aggps[:],
                            lhsT=oh[:, t * 128:(t + 1) * 128],
                            rhs=msg[:, t * c:(t + 1) * c],
                            start=(t == 0), stop=(t == TPB - 1),
                        )
                    agg_sb = spool.tile([128, c], f32, name=f"agg_sb{l}_{b}", tag="aggsb")
                    nc.vector.tensor_copy(agg_sb[:], aggps[:])
                    if debug:
                        nc.sync.dma_start(
                            dbg_d[f"dbg_agg{l}"].ap()[b * 128:(b + 1) * 128, :], agg_sb[:])
                    aggT = ppool.tile([c, 128], f32, name=f"aggT{l}_{b}", tag="aggT", bufs=2)
                    nc.tensor.transpose(out=aggT[:], in_=agg_sb[:], identity=ident_sb[:])
                    zps = ppool.tile([c, 128], f32, name=f"zps{l}_{b}", tag="z", bufs=2)
                    nc.tensor.matmul(
                        out=zps[:, :cnt],
                        lhsT=wroot_sb[l][:],
                        rhs=xT_cur[:, b * 128:b * 128 + cnt],
                        start=True, stop=True,
                    )
                    aggT_sb = spool.tile([c, 128], f32, name=f"aggT_sb{l}_{b}", tag="aggTsb")
                    nc.vector.tensor_copy(aggT_sb[:], aggT[:])
                    z_sb = spool.tile([c, 128], f32, name=f"z_sb{l}_{b}", tag="zsb")
                    nc.vector.tensor_add(out=z_sb[:, :cnt], in0=zps[:, :cnt], in1=aggT_sb[:, :cnt])
                    if l < L - 1:
                        nc.scalar.activation(
                            out=xT_next[:, b * 128:b * 128 + cnt],
                            in_=z_sb[:, :cnt],
                            func=mybir.ActivationFunctionType.Relu,
                            bias=brel_sb[l][:],
                        )
                    else:
                        r_sb = spool.tile([c, 128], f32, name=f"r_sb{b}", tag="rsb")
                        nc.scalar.activation(
                            out=r_sb[:, :cnt], in_=z_sb[:, :cnt],
                            func=mybir.ActivationFunctionType.Relu,
                            bias=brel_sb[l][:],
                        )
                        tps = ppool.tile([128, c], f32, name=f"tps{b}", tag="y", bufs=2)
                        nc.tensor.transpose(
                            out=tps[:cnt, :], in_=r_sb[:, :cnt], identity=ident_sb[:c, :c],
                        )
                        zb = spool.tile([128, c], f32, name=f"zb{b}", tag="zb")
                        nc.vector.tensor_copy(zb[:cnt, :], tps[:cnt, :])
                        mx = spool.tile([128, 1], f32, name=f"mx{b}", tag="mx")
                        nc.vector.tensor_tensor(
                            out=mx[:cnt, :], in0=zb[:cnt, 0:1], in1=zb[:cnt, 1:2],
                            op=mybir.AluOpType.max,
                        )
                        zs = spool.tile([128, c], f32, name=f"zs{b}", tag="zs")
                        nc.vector.tensor_tensor(
                            out=zs[:cnt, :], in0=zb[:cnt, :],
                            in1=mx[:cnt, :].to_broadcast([cnt, c]),
                            op=mybir.AluOpType.subtract,
                        )
                        esb = spool.tile([128, c], f32, name=f"esb{b}", tag="esb")
                        nc.scalar.activation(
                            out=esb[:cnt, :], in_=zs[:cnt, :],
                            func=mybir.ActivationFunctionType.Exp,
                        )
                        ssb = spool.tile([128, 1], f32, name=f"ssb{b}", tag="ssb")
                        nc.vector.tensor_add(out=ssb[:cnt, :], in0=esb[:cnt, 0:1], in1=esb[:cnt, 1:2])
                        rcp = spool.tile([128, 1], f32, name=f"rcp{b}", tag="rcp")
                        nc.vector.reciprocal(rcp[:cnt, :], ssb[:cnt, :])
                        osb = spool.tile([128, c], f32, name=f"osb{b}", tag="osb")
                        nc.vector.tensor_tensor(
                            out=osb[:cnt, :], in0=esb[:cnt, :],
                            in1=rcp[:cnt, :].to_broadcast([cnt, c]),
                            op=mybir.AluOpType.mult,
                        )
                        nc.sync.dma_start(out_d.ap()[b * 128:b * 128 + cnt, :], osb[:cnt, :])
                if debug:
                    if l == 0:
                        nc.sync.dma_start(dbg_d[f"dbg_ytab{l}"].ap()[:], x64_d.ap()[:, :dims[1]])
                    else:
                        nc.sync.dma_start(dbg_d[f"dbg_ytab{l}"].ap()[:], ytabc[:])
                    if l < L - 1:
                        nc.sync.dma_start(dbg_d[f"dbg_xT{l + 1}"].ap()[:], xT_next[:])
                xT_cur = xT_next


def make_host_inputs(inputs, N, dims, ncores):
    """From problem inputs, build per-core in_maps for run_bass_kernel_spmd."""
    x = np.asarray(inputs["x"], np.float32)
    idx16, dstl, wgt, T_chunk, NB, NPC, last_cnt, CH = preprocess(
        inputs["edge_index"], inputs["edge_weight"], N, ncores)
    iota = np.broadcast_to(np.arange(128, dtype=np.float16), (128, 128)).copy()
    ident = np.eye(128, dtype=np.float32)
    # layer-0 gather table: y0 = x @ W_rel0, padded to 256B rows
    x64 = np.zeros((N, PAD), np.float32)
    x64[:, :dims[1]] = x @ np.asarray(inputs["w_rel0"], np.float32)
    L = len(dims) - 1
    common = {"iota": iota, "ident": ident, "x64": x64}
    for l in range(L):
        common[f"wrel{l}"] = np.asarray(inputs[f"w_rel{l}"], np.float32)
        common[f"wroot{l}"] = np.asarray(inputs[f"w_root{l}"], np.float32)
        common[f"brel{l}"] = np.asarray(inputs[f"b_rel{l}"], np.float32).reshape(-1, 1)
    in_maps = []
    for k in range(ncores):
        m = dict(common)
        m["idx16"] = idx16[k]
        m["dstl"] = dstl[k]
        m["wgt"] = wgt[k]
        m["xT0"] = np.ascontiguousarray(x[k * NPC:(k + 1) * NPC].T)
        in_maps.append(m)
    return in_maps, T_chunk, NB, NPC, last_cnt, CH


def _install_loud_hook():
    import traceback
    from concourse import bass2jax
    bass2jax.install_neuronx_cc_hook()
    try:
        import libneuronxla
    except ImportError:
        return
    hook = libneuronxla.neuronx_cc
    def loud(*a, **k):
        try:
            return hook(*a, **k)
        except BaseException:
            traceback.print_exc()
            raise
    libneuronxla.neuronx_cc = loud
    bass2jax.install_neuronx_cc_hook = lambda: None


def run_gnn(inputs, N, dims, ncores=8, trace=False, debug=False):
    from concourse.bass_utils import run_bass_kernel_spmd
    _install_loud_hook()
    in_maps, T_chunk, NB, NPC, last_cnt, CH = make_host_inputs(inputs, N, dims, ncores)
    nc = bacc.Bacc("TRN2", target_bir_lowering=False, debug=False, num_devices=ncores)
    build_gnn(nc, N, NPC, NB, T_chunk, dims, ncores, last_cnt, CH, debug=debug)
    nc.compile()
    res = run_bass_kernel_spmd(nc, in_maps, core_ids=list(range(ncores)), trace=trace)
    out = np.concatenate([res.results[k]["out"] for k in range(ncores)], axis=0)
    return out, res


DIMS = [6, 20, 15, 10, 5, 2]
N_NODES = 100000
N_CORES = 8


def kernel(**inputs):
    out, _res = run_gnn(inputs, N_NODES, DIMS, ncores=N_CORES, trace=False)
    return out


def kernel_traced(**inputs):
    """Like kernel() but also returns the BassKernelResults (exec_time_ns etc)."""
    return run_gnn(inputs, N_NODES, DIMS, ncores=N_CORES, trace=True)



# revision 10
# speedup vs baseline: 1.6009x; 1.6009x over previous
"""Self-contained Trainium2 Bass kernel for the 5-layer GraphConv GNN
(N=100000 nodes, E=3200000 edges, dims 6->20->15->10->5->2, relu, softmax).

kernel(**inputs) takes the full unsharded inputs (as from setup_inputs()),
shards edges across 8 NeuronCores by destination-node range internally,
runs the Bass program via run_bass_kernel_spmd, and returns the full
[100000, 2] float32 output.
"""

import sys
sys.path.insert(0, '/opt/trn_rl_repo')
import numpy as np
import concourse.bass as bass
import concourse.bacc as bacc
import concourse.tile as tile
from concourse import mybir

f32 = mybir.dt.float32
f16 = mybir.dt.float16
i32 = mybir.dt.int32
i16 = mybir.dt.int16

NCHUNK = 4
PAD = 64  # table row padded to 64 f32 = 256 bytes (dma_gather elem quantum)


def preprocess(edge_index, edge_weight, N, ncores):
    """Group edges by (core, dst-block, src-chunk); pad each (block, chunk)
    group to T_chunk tiles of 128 slots (dummy slots: src 0, w 0).

    Slot (b, s) with s = t*128 + p lives at array position [p, b*TPB + t]
    where TPB = NCHUNK*T_chunk tiles per block. Chunk c owns tiles
    [c*T_chunk, (c+1)*T_chunk).

    Returns per-core arrays:
      idx16 [ncores, NB, 128, NCHUNK*T_chunk*8] int16 (gather indices,
            wrapped j%16 -> partition, j//16 -> column, replicated x8)
      dstl  [ncores, 128, NB*TPB] f16
      wgt   [ncores, 128, NB*TPB] f32
    plus (T_chunk, NB, NPC, last_cnt, CH).
    """
    src = np.asarray(edge_index[0], dtype=np.int64)
    dst = np.asarray(edge_index[1], dtype=np.int64)
    w = np.asarray(edge_weight, dtype=np.float32)
    NPC = N // ncores
    NB = (NPC + 127) // 128
    last_cnt = NPC - (NB - 1) * 128
    CH = (N + NCHUNK - 1) // NCHUNK

    core = dst // NPC
    r = dst % NPC
    blk = r // 128
    loc = r % 128
    gb = core * NB + blk
    chunk = src // CH
    srcl = src - chunk * CH
    key = gb * NCHUNK + chunk
    order = np.argsort(key, kind='stable')
    srcl_s, loc_s, w_s, key_s = srcl[order], loc[order], w[order], key[order]

    ngroups = ncores * NB * NCHUNK
    counts = np.bincount(key_s, minlength=ngroups)
    T_chunk = max(1, int(np.ceil(counts.max() / 128)))
    SP = T_chunk * 128            # slots per (block, chunk) group
    S = NCHUNK * SP               # slots per block
    TPB = NCHUNK * T_chunk        # tiles per block

    nblocks = ncores * NB
    # Slot layout per (block, chunk) group: [0:cnt) real edges, [cnt:gmax)
    # dummy idx 0 / w 0 (gmax = max count over cores, so num_idxs_reg=gmax
    # can be baked into the uniform SPMD program), [gmax:SP) idx -1 -- those
    # trailing descriptors are skipped by the DMA.
    srcl_p = np.full((ngroups, SP), -1, np.int16)
    loc_p = np.zeros((ngroups, SP), np.float16)
    w_p = np.zeros((ngroups, SP), np.float32)
    starts = np.concatenate([[0], np.cumsum(counts)[:-1]])
    pos = np.arange(len(key_s)) - starts[key_s]
    srcl_p[key_s, pos] = srcl_s
    loc_p[key_s, pos] = loc_s
    w_p[key_s, pos] = w_s
    gcounts = counts.reshape(ncores, NB * NCHUNK).max(axis=0)  # [NB*NCHUNK]
    for k in range(ncores):
        for g in range(NB * NCHUNK):
            row = k * NB * NCHUNK + g
            cnt = counts[row]
            srcl_p[row, cnt:gcounts[g]] = 0

    # idx16: per (block, chunk) instruction, index j (= slot within group)
    # goes to [j % 16, j // 16]; replicate the 16 rows x8 to 128 partitions.
    idx_wrap = np.ascontiguousarray(
        srcl_p.reshape(ngroups, SP // 16, 16).transpose(0, 2, 1))  # [g, 16, SP/16]
    idx_wrap = np.tile(idx_wrap, (1, 8, 1))  # [g, 128, SP/16]
    idx16 = idx_wrap.reshape(ncores, NB, NCHUNK, 128, SP // 16)
    idx16 = np.ascontiguousarray(
        idx16.transpose(0, 1, 3, 2, 4).reshape(ncores, NB, 128, NCHUNK * (SP // 16)))

    def to_sbuf(a):
        # [ngroups, SP] -> [ncores, 128, NB*TPB] with [p, b*TPB+t] = slot t*128+p
        a = a.reshape(ncores, NB, NCHUNK, T_chunk, 128)
        return np.ascontiguousarray(
            a.transpose(0, 4, 1, 2, 3).reshape(ncores, 128, NB * TPB))

    gcounts = gcounts.reshape(NB, NCHUNK)
    return (idx16, to_sbuf(loc_p), to_sbuf(w_p), T_chunk, NB, NPC, last_cnt, CH,
            gcounts)


def build_gnn(nc, N, NPC, NB, T_chunk, dims, ncores, last_cnt, CH, gcounts=None,
              debug=False):
    """Emit the full 5-layer program into nc."""
    L = len(dims) - 1
    TPB = NCHUNK * T_chunk
    SP = T_chunk * 128
    NQ = nc.num_swdge_queues
    qctr = [0]
    dbg_d = {}
    if debug:
        for l in range(L):
            dbg_d[f"dbg_ytab{l}"] = nc.dram_tensor(
                f"dbg_ytab{l}", [N, dims[l + 1]], f32, kind="ExternalOutput")
            if l < L - 1:
                dbg_d[f"dbg_xT{l + 1}"] = nc.dram_tensor(
                    f"dbg_xT{l + 1}", [dims[l + 1], NPC], f32, kind="ExternalOutput")
            dbg_d[f"dbg_agg{l}"] = nc.dram_tensor(
                f"dbg_agg{l}", [NB * 128, dims[l + 1]], f32, kind="ExternalOutput")

    # ---- DRAM I/O ----
    idx_d = nc.dram_tensor("idx16", [NB, 128, NCHUNK * (SP // 16)], i16, kind="ExternalInput")
    dstl_d = nc.dram_tensor("dstl", [128, NB * TPB], f16, kind="ExternalInput")
    wgt_d = nc.dram_tensor("wgt", [128, NB * TPB], f32, kind="ExternalInput")
    xT0_d = nc.dram_tensor("xT0", [dims[0], NPC], f32, kind="ExternalInput")
    x64_d = nc.dram_tensor("x64", [N, PAD], f32, kind="ExternalInput")
    iota_d = nc.dram_tensor("iota", [128, 128], f16, kind="ExternalInput")
    ident_d = nc.dram_tensor("ident", [128, 128], f32, kind="ExternalInput")
    wrel_d = [nc.dram_tensor(f"wrel{l}", [dims[l], dims[l + 1]], f32, kind="ExternalInput") for l in range(L)]
    wroot_d = [nc.dram_tensor(f"wroot{l}", [dims[l], dims[l + 1]], f32, kind="ExternalInput") for l in range(L)]
    brel_d = [nc.dram_tensor(f"brel{l}", [dims[l + 1], 1], f32, kind="ExternalInput") for l in range(L)]
    out_d = nc.dram_tensor("out", [NPC, dims[L]], f32, kind="ExternalOutput")

    groups = [list(range(ncores))]

    with tile.TileContext(nc) as tc:
        with (
            tc.tile_pool(name="const", bufs=1) as cpool,
            tc.tile_pool(name="edge", bufs=1) as epool,
            tc.tile_pool(name="xts", bufs=1) as xpool,
            tc.tile_pool(name="idxp", bufs=3) as ipool,
            tc.tile_pool(name="gath", bufs=3) as gpool,
            tc.tile_pool(name="msg", bufs=2) as mpool,
            tc.tile_pool(name="ohp", bufs=2) as opool,
            tc.tile_pool(name="small", bufs=3) as spool,
            tc.tile_pool(name="psum", bufs=1, space="PSUM") as ppool,
            tc.tile_pool(name="dram", bufs=1, space="DRAM") as dpool,
        ):
            # ---- load constants / edge data ----
            dstl_sb = epool.tile([128, NB * TPB], f16)
            nc.sync.dma_start(dstl_sb[:], dstl_d.ap()[:])
            wgt_sb = epool.tile([128, NB * TPB], f32)
            nc.sync.dma_start(wgt_sb[:], wgt_d.ap()[:])
            iota_sb = cpool.tile([128, 128], f16)
            nc.sync.dma_start(iota_sb[:], iota_d.ap()[:])
            ident_sb = cpool.tile([128, 128], f32)
            nc.sync.dma_start(ident_sb[:], ident_d.ap()[:])
            wrel_sb, wroot_sb, brel_sb = [], [], []
            for l in range(L):
                t1 = cpool.tile([dims[l], dims[l + 1]], f32, name=f"wrel_sb{l}")
                nc.sync.dma_start(t1[:], wrel_d[l].ap()[:])
                wrel_sb.append(t1)
                t2 = cpool.tile([dims[l], dims[l + 1]], f32, name=f"wroot_sb{l}")
                nc.sync.dma_start(t2[:], wroot_d[l].ap()[:])
                wroot_sb.append(t2)
                t3 = cpool.tile([dims[l + 1], 1], f32, name=f"brel_sb{l}")
                nc.sync.dma_start(t3[:], brel_d[l].ap()[:])
                brel_sb.append(t3)

            xT_cur = xpool.tile([dims[0], NPC], f32, name="xT_l0", tag="xT", bufs=2)
            nc.sync.dma_start(xT_cur[:], xT0_d.ap()[:])

            for l in range(L):
                c = dims[l + 1]
                din = dims[l]
                if l == 0:
                    ytab64 = x64_d.ap()
                else:
                    # ---- y = x @ W_rel locally, AllGather, restride to 256B rows
                    ybounce = dpool.tile([NPC, c], f32, name=f"ybounce{l}")
                    for b in range(NB):
                        cnt = 128 if b < NB - 1 else last_cnt
                        yps = ppool.tile([128, c], f32, name=f"yps{l}_{b}", tag="y", bufs=2)
                        nc.tensor.matmul(
                            out=yps[:cnt, :],
                            lhsT=xT_cur[:, b * 128:b * 128 + cnt],
                            rhs=wrel_sb[l][:],
                            start=True, stop=True,
                        )
                        ysb = spool.tile([128, c], f32, name=f"ysb{l}_{b}", tag="ysb")
                        nc.vector.tensor_copy(ysb[:cnt, :], yps[:cnt, :])
                        nc.sync.dma_start(ybounce[b * 128:b * 128 + cnt, :], ysb[:cnt, :])
                    ytabc = dpool.tile([N, c], f32, name=f"ytabc{l}")
                    nc.gpsimd.collective_compute(
                        "AllGather",
                        mybir.AluOpType.bypass,
                        replica_groups=groups,
                        ins=[ybounce[:].opt()],
                        outs=[ytabc[:].opt()],
                    )
                    ytab64_t = dpool.tile([N, PAD], f32, name=f"ytab64_{l}")
                    for ch in range(NCHUNK):
                        r0, r1 = ch * CH, min((ch + 1) * CH, N)
                        nc.sync.dma_start(ytab64_t[r0:r1, :c], ytabc[r0:r1, :])
                    ytab64 = ytab64_t

                # ---- main per-block loop ----
                if l < L - 1:
                    xT_next = xpool.tile([c, NPC], f32, name=f"xT_l{l + 1}", tag="xT", bufs=2)
                else:
                    xT_next = None
                for b in range(NB):
                    cnt = 128 if b < NB - 1 else last_cnt
                    es = slice(b * TPB, (b + 1) * TPB)
                    idx_sb = ipool.tile([128, NCHUNK * (SP // 16)], i16, name=f"idx{l}_{b}", tag="idx")
                    nc.sync.dma_start(idx_sb[:], idx_d.ap()[b])
                    gth = gpool.tile([128, TPB * PAD], f32, name=f"gth{l}_{b}", tag="gth")
                    if l == 0 and b < 3:
                        # first-touch each of the 3 ring bufs: slots skipped by
                        # short num_idxs_reg must hold finite data (w=0 kills
                        # their contribution, but NaN*0 would not be 0)
                        nc.gpsimd.memset(gth[:], 0)
                    for ch in range(NCHUNK):
                        r0 = ch * CH
                        r1 = min(r0 + CH, N)
                        gmax = SP if gcounts is None else int(gcounts[b][ch])
                        t0 = 0
                        while t0 < T_chunk:
                            tn = min(8, T_chunk - t0)  # dma_gather max 1024 idxs/instr
                            reg = max(0, min(gmax - t0 * 128, tn * 128))
                            if reg == 0:
                                break
                            nc.gpsimd.dma_gather(
                                out_ap=gth[:, (ch * T_chunk + t0) * PAD:(ch * T_chunk + t0 + tn) * PAD]
                                    .rearrange("p (t e) -> p t e", e=PAD),
                                in_ap=ytab64[r0:r1, :],
                                idxs_ap=idx_sb[:, ch * (SP // 16) + t0 * 8:ch * (SP // 16) + (t0 + tn) * 8],
                                num_idxs=tn * 128,
                                num_idxs_reg=reg,
                                elem_size=PAD,
                                queue_num=qctr[0] % NQ,
                            )
                            qctr[0] += 1
                            t0 += tn
                    msg = mpool.tile([128, TPB * c], f16, name=f"msg{l}_{b}", tag="msg")
                    nc.vector.tensor_tensor(
                        out=msg[:].rearrange("p (t c) -> p t c", c=c),
                        in0=gth[:].rearrange("p (t e) -> p t e", e=PAD)[:, :, :c],
                        in1=wgt_sb[:, es].to_broadcast([128, TPB, c]),
                        op=mybir.AluOpType.mult,
                    )
                    oh = opool.tile([128, TPB * 128], f16, name=f"oh{l}_{b}", tag="oh")
                    nc.vector.tensor_tensor(
                        out=oh[:].rearrange("p (t n) -> p t n", n=128),
                        in0=dstl_sb[:, es].to_broadcast([128, TPB, 128]),
                        in1=iota_sb[:, None, :].to_broadcast([128, TPB, 128]),
                        op=mybir.AluOpType.is_equal,
                    )
                    aggps = ppool.tile([128, c], f32, name=f"aggps{l}_{b}", tag="agg", bufs=2)
                    for t in range(TPB):
                        nc.tensor.matmul(
                            out=aggps[:],
                            lhsT=oh[:, t * 128:(t + 1) * 128],
                            rhs=msg[:, t * c:(t + 1) * c],
                            start=(t == 0), stop=(t == TPB - 1),
                        )
                    agg_sb = spool.tile([128, c], f32, name=f"agg_sb{l}_{b}", tag="aggsb")
                    nc.vector.tensor_copy(agg_sb[:], aggps[:])
                    if debug:
                        nc.sync.dma_start(
                            dbg_d[f"dbg_agg{l}"].ap()[b * 128:(b + 1) * 128, :], agg_sb[:])
                    aggT = ppool.tile([c, 128], f32, name=f"aggT{l}_{b}", tag="aggT", bufs=2)
                    nc.tensor.transpose(out=aggT[:], in_=agg_sb[:], identity=ident_sb[:])
                    zps = ppool.tile([c, 128], f32, name=f"zps{l}_{b}", tag="z", bufs=2)
                    nc.tensor.matmul(
                        out=zps[:, :cnt],
                        lhsT=wroot_sb[l][:],
                        rhs=xT_cur[:, b * 128:b * 128 + cnt],
                        start=True, stop=True,
                    )
                    aggT_sb = spool.tile([c, 128], f32, name=f"aggT_sb{l}_{b}", tag="aggTsb")
                    nc.vector.tensor_copy(aggT_sb[:], aggT[:])
                    z_sb = spool.tile([c, 128], f32, name=f"z_sb{l}_{b}", tag="zsb")
                    nc.vector.tensor_add(out=z_sb[:, :cnt], in0=zps[:, :cnt], in1=aggT_sb[:, :cnt])
                    if l < L - 1:
                        nc.scalar.activation(
                            out=xT_next[:, b * 128:b * 128 + cnt],
                            in_=z_sb[:, :cnt],
                            func=mybir.ActivationFunctionType.Relu,
                            bias=brel_sb[l][:],
                        )
                    else:
                        r_sb = spool.tile([c, 128], f32, name=f"r_sb{b}", tag="rsb")
                        nc.scalar.activation(
                            out=r_sb[:, :cnt], in_=z_sb[:, :cnt],
                            func=mybir.ActivationFunctionType.Relu,
                            bias=brel_sb[l][:],
                        )
                        tps = ppool.tile([128, c], f32, name=f"tps{b}", tag="y", bufs=2)
                        nc.tensor.transpose(
                            out=tps[:cnt, :], in_=r_sb[:, :cnt], identity=ident_sb[:c, :c],
                        )
                        zb = spool.tile([128, c], f32, name=f"zb{b}", tag="zb")
                        nc.vector.tensor_copy(zb[:cnt, :], tps[:cnt, :])
                        mx = spool.tile([128, 1], f32, name=f"mx{b}", tag="mx")
                        nc.vector.tensor_tensor(
                            out=mx[:cnt, :], in0=zb[:cnt, 0:1], in1=zb[:cnt, 1:2],
                            op=mybir.AluOpType.max,
                        )
                        zs = spool.tile([128, c], f32, name=f"zs{b}", tag="zs")
                        nc.vector.tensor_tensor(
                            out=zs[:cnt, :], in0=zb[:cnt, :],
                            in1=mx[:cnt, :].to_broadcast([cnt, c]),
                            op=mybir.AluOpType.subtract,
                        )
                        esb = spool.tile([128, c], f32, name=f"esb{b}", tag="esb")
                        nc.scalar.activation(
                            out=esb[:cnt, :], in_=zs[:cnt, :],
                            func=mybir.ActivationFunctionType.Exp,
                        )
                        ssb = spool.tile([128, 1], f32, name=f"ssb{b}", tag="ssb")
                        nc.vector.tensor_add(out=ssb[:cnt, :], in0=esb[:cnt, 0:1], in1=esb[:cnt, 1:2])
                        rcp = spool.tile([128, 1], f32, name=f"rcp{b}", tag="rcp")
                        nc.vector.reciprocal(rcp[:cnt, :], ssb[:cnt, :])
                        osb = spool.tile([128, c], f32, name=f"osb{b}", tag="osb")
                        nc.vector.tensor_tensor(
                            out=osb[:cnt, :], in0=esb[:cnt, :],
                            in1=rcp[:cnt, :].to_broadcast([cnt, c]),
                            op=mybir.AluOpType.mult,
                        )
                        nc.sync.dma_start(out_d.ap()[b * 128:b * 128 + cnt, :], osb[:cnt, :])
                if debug:
                    if l == 0:
                        nc.sync.dma_start(dbg_d[f"dbg_ytab{l}"].ap()[:], x64_d.ap()[:, :dims[1]])
                    else:
                        nc.sync.dma_start(dbg_d[f"dbg_ytab{l}"].ap()[:], ytabc[:])
                    if l < L - 1:
                        nc.sync.dma_start(dbg_d[f"dbg_xT{l + 1}"].ap()[:], xT_next[:])
                xT_cur = xT_next


def make_host_inputs(inputs, N, dims, ncores):
    """From problem inputs, build per-core in_maps for run_bass_kernel_spmd."""
    x = np.asarray(inputs["x"], np.float32)
    (idx16, dstl, wgt, T_chunk, NB, NPC, last_cnt, CH, gcounts) = preprocess(
        inputs["edge_index"], inputs["edge_weight"], N, ncores)
    iota = np.broadcast_to(np.arange(128, dtype=np.float16), (128, 128)).copy()
    ident = np.eye(128, dtype=np.float32)
    # layer-0 gather table: y0 = x @ W_rel0, padded to 256B rows
    x64 = np.zeros((N, PAD), np.float32)
    x64[:, :dims[1]] = x @ np.asarray(inputs["w_rel0"], np.float32)
    L = len(dims) - 1
    common = {"iota": iota, "ident": ident, "x64": x64}
    for l in range(L):
        common[f"wrel{l}"] = np.asarray(inputs[f"w_rel{l}"], np.float32)
        common[f"wroot{l}"] = np.asarray(inputs[f"w_root{l}"], np.float32)
        common[f"brel{l}"] = np.asarray(inputs[f"b_rel{l}"], np.float32).reshape(-1, 1)
    in_maps = []
    for k in range(ncores):
        m = dict(common)
        m["idx16"] = idx16[k]
        m["dstl"] = dstl[k]
        m["wgt"] = wgt[k]
        m["xT0"] = np.ascontiguousarray(x[k * NPC:(k + 1) * NPC].T)
        in_maps.append(m)
    return in_maps, T_chunk, NB, NPC, last_cnt, CH, gcounts


def _install_loud_hook():
    import traceback
    from concourse import bass2jax
    bass2jax.install_neuronx_cc_hook()
    try:
        import libneuronxla
    except ImportError:
        return
    hook = libneuronxla.neuronx_cc
    def loud(*a, **k):
        try:
            return hook(*a, **k)
        except BaseException:
            traceback.print_exc()
            raise
    libneuronxla.neuronx_cc = loud
    bass2jax.install_neuronx_cc_hook = lambda: None


def run_gnn(inputs, N, dims, ncores=8, trace=False, debug=False):
    from concourse.bass_utils import run_bass_kernel_spmd
    _install_loud_hook()
    in_maps, T_chunk, NB, NPC, last_cnt, CH, gcounts = make_host_inputs(
        inputs, N, dims, ncores)
    nc = bacc.Bacc("TRN2", target_bir_lowering=False, debug=False,
                   num_devices=ncores, num_swdge_queues=4)
    build_gnn(nc, N, NPC, NB, T_chunk, dims, ncores, last_cnt, CH,
              gcounts=gcounts, debug=debug)
    nc.compile()
    res = run_bass_kernel_spmd(nc, in_maps, core_ids=list(range(ncores)), trace=trace)
    out = np.concatenate([res.results[k]["out"] for k in range(ncores)], axis=0)
    return out, res


DIMS = [6, 20, 15, 10, 5, 2]
N_NODES = 100000
N_CORES = 8


def kernel(**inputs):
    out, _res = run_gnn(inputs, N_NODES, DIMS, ncores=N_CORES, trace=False)
    return out


def kernel_traced(**inputs):
    """Like kernel() but also returns the BassKernelResults (exec_time_ns etc)."""
    return run_gnn(inputs, N_NODES, DIMS, ncores=N_CORES, trace=True)



# revision 16
# speedup vs baseline: 1.7071x; 1.0663x over previous
"""Self-contained Trainium2 Bass kernel for the 5-layer GraphConv GNN
(N=100000 nodes, E=3200000 edges, dims 6->20->15->10->5->2, relu, softmax).

kernel(**inputs) takes the full unsharded inputs (as from setup_inputs()),
shards edges across 8 NeuronCores by destination-node range internally,
runs the Bass program via run_bass_kernel_spmd, and returns the full
[100000, 2] float32 output.
"""

import sys
sys.path.insert(0, '/opt/trn_rl_repo')
import numpy as np
import concourse.bass as bass
import concourse.bacc as bacc
import concourse.tile as tile
from concourse import mybir

f32 = mybir.dt.float32
f16 = mybir.dt.float16
i32 = mybir.dt.int32
i16 = mybir.dt.int16

NCHUNK = 4
PAD = 64  # table row padded to 64 f32 = 256 bytes (dma_gather elem quantum)


def preprocess(edge_index, edge_weight, N, ncores):
    """Group edges by (core, dst-block, src-chunk); pad each (block, chunk)
    group to T_chunk tiles of 128 slots (dummy slots: src 0, w 0).

    Slot (b, s) with s = t*128 + p lives at array position [p, b*TPB + t]
    where TPB = NCHUNK*T_chunk tiles per block. Chunk c owns tiles
    [c*T_chunk, (c+1)*T_chunk).

    Returns per-core arrays:
      idx16 [ncores, NB, 128, NCHUNK*T_chunk*8] int16 (gather indices,
            wrapped j%16 -> partition, j//16 -> column, replicated x8)
      dstl  [ncores, 128, NB*TPB] f16
      wgt   [ncores, 128, NB*TPB] f32
    plus (T_chunk, NB, NPC, last_cnt, CH).
    """
    src = np.asarray(edge_index[0], dtype=np.int64)
    dst = np.asarray(edge_index[1], dtype=np.int64)
    w = np.asarray(edge_weight, dtype=np.float32)
    NPC = N // ncores
    NB = (NPC + 127) // 128
    last_cnt = NPC - (NB - 1) * 128
    CH = (N + NCHUNK - 1) // NCHUNK

    core = dst // NPC
    r = dst % NPC
    blk = r // 128
    loc = r % 128
    gb = core * NB + blk
    chunk = src // CH
    srcl = src - chunk * CH
    key = gb * NCHUNK + chunk
    order = np.argsort(key, kind='stable')
    srcl_s, loc_s, w_s, key_s = srcl[order], loc[order], w[order], key[order]

    ngroups = ncores * NB * NCHUNK
    counts = np.bincount(key_s, minlength=ngroups)
    T_chunk = max(1, int(np.ceil(counts.max() / 128)))
    SP = T_chunk * 128            # slots per (block, chunk) group
    S = NCHUNK * SP               # slots per block
    TPB = NCHUNK * T_chunk        # tiles per block

    nblocks = ncores * NB
    # Slot layout per (block, chunk) group: [0:cnt) real edges, [cnt:gmax)
    # dummy idx 0 / w 0 (gmax = max count over cores, so num_idxs_reg=gmax
    # can be baked into the uniform SPMD program), [gmax:SP) idx -1 -- those
    # trailing descriptors are skipped by the DMA.
    srcl_p = np.full((ngroups, SP), -1, np.int16)
    loc_p = np.zeros((ngroups, SP), np.float16)
    w_p = np.zeros((ngroups, SP), np.float32)
    starts = np.concatenate([[0], np.cumsum(counts)[:-1]])
    pos = np.arange(len(key_s)) - starts[key_s]
    srcl_p[key_s, pos] = srcl_s
    loc_p[key_s, pos] = loc_s
    w_p[key_s, pos] = w_s
    gcounts = counts.reshape(ncores, NB * NCHUNK).max(axis=0)  # [NB*NCHUNK]
    for k in range(ncores):
        for g in range(NB * NCHUNK):
            row = k * NB * NCHUNK + g
            cnt = counts[row]
            srcl_p[row, cnt:gcounts[g]] = 0

    # idx16: per (block, chunk) instruction, index j (= slot within group)
    # goes to [j % 16, j // 16]; replicate the 16 rows x8 to 128 partitions.
    idx_wrap = np.ascontiguousarray(
        srcl_p.reshape(ngroups, SP // 16, 16).transpose(0, 2, 1))  # [g, 16, SP/16]
    idx_wrap = np.tile(idx_wrap, (1, 8, 1))  # [g, 128, SP/16]
    idx16 = idx_wrap.reshape(ncores, NB, NCHUNK, 128, SP // 16)
    idx16 = np.ascontiguousarray(
        idx16.transpose(0, 1, 3, 2, 4).reshape(ncores, NB, 128, NCHUNK * (SP // 16)))

    def to_sbuf(a):
        # [ngroups, SP] -> [ncores, 128, NB*TPB] with [p, b*TPB+t] = slot t*128+p
        a = a.reshape(ncores, NB, NCHUNK, T_chunk, 128)
        return np.ascontiguousarray(
            a.transpose(0, 4, 1, 2, 3).reshape(ncores, 128, NB * TPB))

    gcounts = gcounts.reshape(NB, NCHUNK)
    return (idx16, to_sbuf(loc_p), to_sbuf(w_p), T_chunk, NB, NPC, last_cnt, CH,
            gcounts)


def build_gnn(nc, N, NPC, NB, T_chunk, dims, ncores, last_cnt, CH, gcounts=None,
              debug=False):
    """Emit the full 5-layer program into nc."""
    L = len(dims) - 1
    TPB = NCHUNK * T_chunk
    SP = T_chunk * 128
    NQ = nc.num_swdge_queues
    qctr = [0]
    dbg_d = {}
    if debug:
        for l in range(L):
            dbg_d[f"dbg_ytab{l}"] = nc.dram_tensor(
                f"dbg_ytab{l}", [N, dims[l + 1]], f32, kind="ExternalOutput")
            if l < L - 1:
                dbg_d[f"dbg_xT{l + 1}"] = nc.dram_tensor(
                    f"dbg_xT{l + 1}", [dims[l + 1], NPC], f32, kind="ExternalOutput")
            dbg_d[f"dbg_agg{l}"] = nc.dram_tensor(
                f"dbg_agg{l}", [NB * 128, dims[l + 1]], f32, kind="ExternalOutput")

    # ---- DRAM I/O ----
    idx_d = nc.dram_tensor("idx16", [NB, 128, NCHUNK * (SP // 16)], i16, kind="ExternalInput")
    dstl_d = nc.dram_tensor("dstl", [128, NB * TPB], f16, kind="ExternalInput")
    wgt_d = nc.dram_tensor("wgt", [128, NB * TPB], f32, kind="ExternalInput")
    xT0_d = nc.dram_tensor("xT0", [dims[0], NPC], f32, kind="ExternalInput")
    x64_d = nc.dram_tensor("x64", [N, PAD], f32, kind="ExternalInput")
    iota_d = nc.dram_tensor("iota", [128, 128], f16, kind="ExternalInput")
    ident_d = nc.dram_tensor("ident", [128, 128], f32, kind="ExternalInput")
    wrel_d = [nc.dram_tensor(f"wrel{l}", [dims[l], dims[l + 1]], f32, kind="ExternalInput") for l in range(L)]
    wroot_d = [nc.dram_tensor(f"wroot{l}", [dims[l], dims[l + 1]], f32, kind="ExternalInput") for l in range(L)]
    brel_d = [nc.dram_tensor(f"brel{l}", [dims[l + 1], 1], f32, kind="ExternalInput") for l in range(L)]
    out_d = nc.dram_tensor("out", [NPC, dims[L]], f32, kind="ExternalOutput")

    groups = [list(range(ncores))]

    with tile.TileContext(nc) as tc:
        with (
            tc.tile_pool(name="const", bufs=1) as cpool,
            tc.tile_pool(name="edge", bufs=1) as epool,
            tc.tile_pool(name="xts", bufs=1) as xpool,
            tc.tile_pool(name="idxp", bufs=4) as ipool,
            tc.tile_pool(name="gath", bufs=4) as gpool,
            tc.tile_pool(name="msg", bufs=3) as mpool,
            tc.tile_pool(name="ohp", bufs=3) as opool,
            tc.tile_pool(name="small", bufs=4) as spool,
            tc.tile_pool(name="psum", bufs=1, space="PSUM") as ppool,
            tc.tile_pool(name="dram", bufs=1, space="DRAM") as dpool,
        ):
            # ---- load constants / edge data ----
            dstl_sb = epool.tile([128, NB * TPB], f16)
            nc.sync.dma_start(dstl_sb[:], dstl_d.ap()[:])
            wgt_sb = epool.tile([128, NB * TPB], f32)
            nc.sync.dma_start(wgt_sb[:], wgt_d.ap()[:])
            iota_sb = cpool.tile([128, 128], f16)
            nc.sync.dma_start(iota_sb[:], iota_d.ap()[:])
            ident_sb = cpool.tile([128, 128], f32)
            nc.sync.dma_start(ident_sb[:], ident_d.ap()[:])
            wrel_sb, wroot_sb, brel_sb = [], [], []
            for l in range(L):
                t1 = cpool.tile([dims[l], dims[l + 1]], f32, name=f"wrel_sb{l}")
                nc.sync.dma_start(t1[:], wrel_d[l].ap()[:])
                wrel_sb.append(t1)
                t2 = cpool.tile([dims[l], dims[l + 1]], f32, name=f"wroot_sb{l}")
                nc.sync.dma_start(t2[:], wroot_d[l].ap()[:])
                wroot_sb.append(t2)
                t3 = cpool.tile([dims[l + 1], 1], f32, name=f"brel_sb{l}")
                nc.sync.dma_start(t3[:], brel_d[l].ap()[:])
                brel_sb.append(t3)

            xT_cur = xpool.tile([dims[0], NPC], f32, name="xT_l0", tag="xT", bufs=2)
            nc.sync.dma_start(xT_cur[:], xT0_d.ap()[:])

            for l in range(L):
                c = dims[l + 1]
                din = dims[l]
                if l == 0:
                    ytab64 = x64_d.ap()
                else:
                    # ---- y = x @ W_rel locally, AllGather, restride to 256B rows
                    ybounce = dpool.tile([NPC, c], f32, name=f"ybounce{l}")
                    for b in range(NB):
                        cnt = 128 if b < NB - 1 else last_cnt
                        yps = ppool.tile([128, c], f32, name=f"yps{l}_{b}", tag="y", bufs=2)
                        nc.tensor.matmul(
                            out=yps[:cnt, :],
                            lhsT=xT_cur[:, b * 128:b * 128 + cnt],
                            rhs=wrel_sb[l][:],
                            start=True, stop=True,
                        )
                        ysb = spool.tile([128, c], f32, name=f"ysb{l}_{b}", tag="ysb")
                        nc.vector.tensor_copy(ysb[:cnt, :], yps[:cnt, :])
                        nc.sync.dma_start(ybounce[b * 128:b * 128 + cnt, :], ysb[:cnt, :])
                    ytabc = dpool.tile([N, c], f32, name=f"ytabc{l}")
                    nc.gpsimd.collective_compute(
                        "AllGather",
                        mybir.AluOpType.bypass,
                        replica_groups=groups,
                        ins=[ybounce[:].opt()],
                        outs=[ytabc[:].opt()],
                    )
                    ytab64_t = dpool.tile([N, PAD], f32, name=f"ytab64_{l}")
                    for ch in range(NCHUNK):
                        r0, r1 = ch * CH, min((ch + 1) * CH, N)
                        nc.sync.dma_start(ytab64_t[r0:r1, :c], ytabc[r0:r1, :])
                    ytab64 = ytab64_t

                # ---- main per-block loop ----
                if l < L - 1:
                    xT_next = xpool.tile([c, NPC], f32, name=f"xT_l{l + 1}", tag="xT", bufs=2)
                else:
                    xT_next = None
                for b in range(NB):
                    cnt = 128 if b < NB - 1 else last_cnt
                    es = slice(b * TPB, (b + 1) * TPB)
                    idx_sb = ipool.tile([128, NCHUNK * (SP // 16)], i16,
                                        name=f"idx{l}_{b}", tag="idx")
                    nc.sync.dma_start(idx_sb[:], idx_d.ap()[b])
                    gth = gpool.tile([128, TPB * PAD], f32, name=f"gth{l}_{b}", tag="gth")
                    if l == 0 and b < 3:
                        # first-touch each of the 3 ring bufs: slots skipped by
                        # short num_idxs_reg must hold finite data (w=0 kills
                        # their contribution, but NaN*0 would not be 0)
                        nc.gpsimd.memset(gth[:], 0)
                    for ch in range(NCHUNK):
                        r0 = ch * CH
                        r1 = min(r0 + CH, N)
                        gmax = SP if gcounts is None else int(gcounts[b][ch])
                        t0 = 0
                        while t0 < T_chunk:
                            tn = min(8, T_chunk - t0)  # dma_gather max 1024 idxs/instr
                            reg = max(0, min(gmax - t0 * 128, tn * 128))
                            if reg == 0:
                                break
                            nc.gpsimd.dma_gather(
                                out_ap=gth[:, (ch * T_chunk + t0) * PAD:(ch * T_chunk + t0 + tn) * PAD]
                                    .rearrange("p (t e) -> p t e", e=PAD),
                                in_ap=ytab64[r0:r1, :],
                                idxs_ap=idx_sb[:, ch * (SP // 16) + t0 * 8:ch * (SP // 16) + (t0 + tn) * 8],
                                num_idxs=tn * 128,
                                num_idxs_reg=reg,
                                elem_size=PAD,
                                queue_num=qctr[0] % NQ,
                            )
                            qctr[0] += 1
                            t0 += tn
                    msg = mpool.tile([128, TPB * c], f16, name=f"msg{l}_{b}", tag="msg")
                    nc.vector.tensor_tensor(
                        out=msg[:].rearrange("p (t c) -> p t c", c=c),
                        in0=gth[:].rearrange("p (t e) -> p t e", e=PAD)[:, :, :c],
                        in1=wgt_sb[:, es].to_broadcast([128, TPB, c]),
                        op=mybir.AluOpType.mult,
                    )
                    oh = opool.tile([128, TPB * 128], f16, name=f"oh{l}_{b}", tag="oh")
                    nc.vector.tensor_tensor(
                        out=oh[:].rearrange("p (t n) -> p t n", n=128),
                        in0=dstl_sb[:, es].to_broadcast([128, TPB, 128]),
                        in1=iota_sb[:, None, :].to_broadcast([128, TPB, 128]),
                        op=mybir.AluOpType.is_equal,
                    )
                    aggps = ppool.tile([128, c], f32, name=f"aggps{l}_{b}", tag="agg", bufs=2)
                    for t in range(TPB):
                        nc.tensor.matmul(
                            out=aggps[:],
                            lhsT=oh[:, t * 128:(t + 1) * 128],
                            rhs=msg[:, t * c:(t + 1) * c],
                            start=(t == 0), stop=(t == TPB - 1),
                        )
                    agg_sb = spool.tile([128, c], f32, name=f"agg_sb{l}_{b}", tag="aggsb")
                    nc.vector.tensor_copy(agg_sb[:], aggps[:])
                    if debug:
                        nc.sync.dma_start(
                            dbg_d[f"dbg_agg{l}"].ap()[b * 128:(b + 1) * 128, :], agg_sb[:])
                    aggT = ppool.tile([c, 128], f32, name=f"aggT{l}_{b}", tag="aggT", bufs=2)
                    nc.tensor.transpose(out=aggT[:], in_=agg_sb[:], identity=ident_sb[:])
                    zps = ppool.tile([c, 128], f32, name=f"zps{l}_{b}", tag="z", bufs=2)
                    nc.tensor.matmul(
                        out=zps[:, :cnt],
                        lhsT=wroot_sb[l][:],
                        rhs=xT_cur[:, b * 128:b * 128 + cnt],
                        start=True, stop=True,
                    )
                    aggT_sb = spool.tile([c, 128], f32, name=f"aggT_sb{l}_{b}", tag="aggTsb")
                    nc.vector.tensor_copy(aggT_sb[:], aggT[:])
                    z_sb = spool.tile([c, 128], f32, name=f"z_sb{l}_{b}", tag="zsb")
                    nc.vector.tensor_add(out=z_sb[:, :cnt], in0=zps[:, :cnt], in1=aggT_sb[:, :cnt])
                    if l < L - 1:
                        nc.scalar.activation(
                            out=xT_next[:, b * 128:b * 128 + cnt],
                            in_=z_sb[:, :cnt],
                            func=mybir.ActivationFunctionType.Relu,
                            bias=brel_sb[l][:],
                        )
                    else:
                        r_sb = spool.tile([c, 128], f32, name=f"r_sb{b}", tag="rsb")
                        nc.scalar.activation(
                            out=r_sb[:, :cnt], in_=z_sb[:, :cnt],
                            func=mybir.ActivationFunctionType.Relu,
                            bias=brel_sb[l][:],
                        )
                        tps = ppool.tile([128, c], f32, name=f"tps{b}", tag="y", bufs=2)
                        nc.tensor.transpose(
                            out=tps[:cnt, :], in_=r_sb[:, :cnt], identity=ident_sb[:c, :c],
                        )
                        zb = spool.tile([128, c], f32, name=f"zb{b}", tag="zb")
                        nc.vector.tensor_copy(zb[:cnt, :], tps[:cnt, :])
                        mx = spool.tile([128, 1], f32, name=f"mx{b}", tag="mx")
                        nc.vector.tensor_tensor(
                            out=mx[:cnt, :], in0=zb[:cnt, 0:1], in1=zb[:cnt, 1:2],
                            op=mybir.AluOpType.max,
                        )
                        zs = spool.tile([128, c], f32, name=f"zs{b}", tag="zs")
                        nc.vector.tensor_tensor(
                            out=zs[:cnt, :], in0=zb[:cnt, :],
                            in1=mx[:cnt, :].to_broadcast([cnt, c]),
                            op=mybir.AluOpType.subtract,
                        )
                        esb = spool.tile([128, c], f32, name=f"esb{b}", tag="esb")
                        nc.scalar.activation(
                            out=esb[:cnt, :], in_=zs[:cnt, :],
                            func=mybir.ActivationFunctionType.Exp,
                        )
                        ssb = spool.tile([128, 1], f32, name=f"ssb{b}", tag="ssb")
                        nc.vector.tensor_add(out=ssb[:cnt, :], in0=esb[:cnt, 0:1], in1=esb[:cnt, 1:2])
                        rcp = spool.tile([128, 1], f32, name=f"rcp{b}", tag="rcp")
                        nc.vector.reciprocal(rcp[:cnt, :], ssb[:cnt, :])
                        osb = spool.tile([128, c], f32, name=f"osb{b}", tag="osb")
                        nc.vector.tensor_tensor(
                            out=osb[:cnt, :], in0=esb[:cnt, :],
                            in1=rcp[:cnt, :].to_broadcast([cnt, c]),
                            op=mybir.AluOpType.mult,
                        )
                        nc.sync.dma_start(out_d.ap()[b * 128:b * 128 + cnt, :], osb[:cnt, :])
                if debug:
                    if l == 0:
                        nc.sync.dma_start(dbg_d[f"dbg_ytab{l}"].ap()[:], x64_d.ap()[:, :dims[1]])
                    else:
                        nc.sync.dma_start(dbg_d[f"dbg_ytab{l}"].ap()[:], ytabc[:])
                    if l < L - 1:
                        nc.sync.dma_start(dbg_d[f"dbg_xT{l + 1}"].ap()[:], xT_next[:])
                xT_cur = xT_next


def make_host_inputs(inputs, N, dims, ncores):
    """From problem inputs, build per-core in_maps for run_bass_kernel_spmd."""
    x = np.asarray(inputs["x"], np.float32)
    (idx16, dstl, wgt, T_chunk, NB, NPC, last_cnt, CH, gcounts) = preprocess(
        inputs["edge_index"], inputs["edge_weight"], N, ncores)
    iota = np.broadcast_to(np.arange(128, dtype=np.float16), (128, 128)).copy()
    ident = np.eye(128, dtype=np.float32)
    # layer-0 gather table: y0 = x @ W_rel0, padded to 256B rows
    x64 = np.zeros((N, PAD), np.float32)
    x64[:, :dims[1]] = x @ np.asarray(inputs["w_rel0"], np.float32)
    L = len(dims) - 1
    common = {"iota": iota, "ident": ident, "x64": x64}
    for l in range(L):
        common[f"wrel{l}"] = np.asarray(inputs[f"w_rel{l}"], np.float32)
        common[f"wroot{l}"] = np.asarray(inputs[f"w_root{l}"], np.float32)
        common[f"brel{l}"] = np.asarray(inputs[f"b_rel{l}"], np.float32).reshape(-1, 1)
    in_maps = []
    for k in range(ncores):
        m = dict(common)
        m["idx16"] = idx16[k]
        m["dstl"] = dstl[k]
        m["wgt"] = wgt[k]
        m["xT0"] = np.ascontiguousarray(x[k * NPC:(k + 1) * NPC].T)
        in_maps.append(m)
    return in_maps, T_chunk, NB, NPC, last_cnt, CH, gcounts


def _install_loud_hook():
    import traceback
    from concourse import bass2jax
    bass2jax.install_neuronx_cc_hook()
    try:
        import libneuronxla
    except ImportError:
        return
    hook = libneuronxla.neuronx_cc
    def loud(*a, **k):
        try:
            return hook(*a, **k)
        except BaseException:
            traceback.print_exc()
            raise
    libneuronxla.neuronx_cc = loud
    bass2jax.install_neuronx_cc_hook = lambda: None


def run_gnn(inputs, N, dims, ncores=8, trace=False, debug=False):
    from concourse.bass_utils import run_bass_kernel_spmd
    _install_loud_hook()
    in_maps, T_chunk, NB, NPC, last_cnt, CH, gcounts = make_host_inputs(
        inputs, N, dims, ncores)
    nc = bacc.Bacc("TRN2", target_bir_lowering=False, debug=False,
                   num_devices=ncores, num_swdge_queues=4)
    build_gnn(nc, N, NPC, NB, T_chunk, dims, ncores, last_cnt, CH,
              gcounts=gcounts, debug=debug)
    nc.compile()
    res = run_bass_kernel_spmd(nc, in_maps, core_ids=list(range(ncores)), trace=trace)
    out = np.concatenate([res.results[k]["out"] for k in range(ncores)], axis=0)
    return out, res


DIMS = [6, 20, 15, 10, 5, 2]
N_NODES = 100000
N_CORES = 8


def kernel(**inputs):
    out, _res = run_gnn(inputs, N_NODES, DIMS, ncores=N_CORES, trace=False)
    return out


def kernel_traced(**inputs):
    """Like kernel() but also returns the BassKernelResults (exec_time_ns etc)."""
    return run_gnn(inputs, N_NODES, DIMS, ncores=N_CORES, trace=True)



# revision 17
# speedup vs baseline: 2.1062x; 1.2338x over previous
"""Self-contained Trainium2 Bass kernel for the 5-layer GraphConv GNN
(N=100000 nodes, E=3200000 edges, dims 6->20->15->10->5->2, relu, softmax).

kernel(**inputs) takes the full unsharded inputs (as from setup_inputs()),
shards edges across 8 NeuronCores by destination-node range internally,
runs the Bass program via run_bass_kernel_spmd, and returns the full
[100000, 2] float32 output.
"""

import sys
sys.path.insert(0, '/opt/trn_rl_repo')
import numpy as np
import concourse.bass as bass
import concourse.bacc as bacc
import concourse.tile as tile
from concourse import mybir

f32 = mybir.dt.float32
f16 = mybir.dt.float16
i32 = mybir.dt.int32
i16 = mybir.dt.int16

NCHUNK = 5
PAD = 64  # table row padded to 64 f32 = 256 bytes (dma_gather elem quantum)


def preprocess(edge_index, edge_weight, N, ncores):
    """Group edges by (core, dst-block, src-chunk); pad each (block, chunk)
    group to T_chunk tiles of 128 slots (dummy slots: src 0, w 0).

    Slot (b, s) with s = t*128 + p lives at array position [p, b*TPB + t]
    where TPB = NCHUNK*T_chunk tiles per block. Chunk c owns tiles
    [c*T_chunk, (c+1)*T_chunk).

    Returns per-core arrays:
      idx16 [ncores, NB, 128, NCHUNK*T_chunk*8] int16 (gather indices,
            wrapped j%16 -> partition, j//16 -> column, replicated x8)
      dstl  [ncores, 128, NB*TPB] f16
      wgt   [ncores, 128, NB*TPB] f32
    plus (T_chunk, NB, NPC, last_cnt, CH).
    """
    src = np.asarray(edge_index[0], dtype=np.int64)
    dst = np.asarray(edge_index[1], dtype=np.int64)
    w = np.asarray(edge_weight, dtype=np.float32)
    NPC = N // ncores
    NB = (NPC + 127) // 128
    last_cnt = NPC - (NB - 1) * 128
    CH = (N + NCHUNK - 1) // NCHUNK

    core = dst // NPC
    r = dst % NPC
    blk = r // 128
    loc = r % 128
    gb = core * NB + blk
    chunk = src // CH
    srcl = src - chunk * CH
    key = gb * NCHUNK + chunk
    order = np.argsort(key, kind='stable')
    srcl_s, loc_s, w_s, key_s = srcl[order], loc[order], w[order], key[order]

    ngroups = ncores * NB * NCHUNK
    counts = np.bincount(key_s, minlength=ngroups)
    T_chunk = max(1, int(np.ceil(counts.max() / 128)))
    SP = T_chunk * 128            # slots per (block, chunk) group
    S = NCHUNK * SP               # slots per block
    TPB = NCHUNK * T_chunk        # tiles per block

    nblocks = ncores * NB
    # Slot layout per (block, chunk) group: [0:cnt) real edges, [cnt:gmax)
    # dummy idx 0 / w 0 (gmax = max count over cores, so num_idxs_reg=gmax
    # can be baked into the uniform SPMD program), [gmax:SP) idx -1 -- those
    # trailing descriptors are skipped by the DMA.
    srcl_p = np.full((ngroups, SP), -1, np.int16)
    loc_p = np.zeros((ngroups, SP), np.float16)
    w_p = np.zeros((ngroups, SP), np.float32)
    starts = np.concatenate([[0], np.cumsum(counts)[:-1]])
    pos = np.arange(len(key_s)) - starts[key_s]
    srcl_p[key_s, pos] = srcl_s
    loc_p[key_s, pos] = loc_s
    w_p[key_s, pos] = w_s
    gcounts = counts.reshape(ncores, NB * NCHUNK).max(axis=0)  # [NB*NCHUNK]
    for k in range(ncores):
        for g in range(NB * NCHUNK):
            row = k * NB * NCHUNK + g
            cnt = counts[row]
            srcl_p[row, cnt:gcounts[g]] = 0

    # idx16: per (block, chunk) instruction, index j (= slot within group)
    # goes to [j % 16, j // 16]; replicate the 16 rows x8 to 128 partitions.
    idx_wrap = np.ascontiguousarray(
        srcl_p.reshape(ngroups, SP // 16, 16).transpose(0, 2, 1))  # [g, 16, SP/16]
    idx_wrap = np.tile(idx_wrap, (1, 8, 1))  # [g, 128, SP/16]
    idx16 = idx_wrap.reshape(ncores, NB, NCHUNK, 128, SP // 16)
    idx16 = np.ascontiguousarray(
        idx16.transpose(0, 1, 3, 2, 4).reshape(ncores, NB, 128, NCHUNK * (SP // 16)))

    def to_sbuf(a):
        # [ngroups, SP] -> [ncores, 128, NB*TPB] with [p, b*TPB+t] = slot t*128+p
        a = a.reshape(ncores, NB, NCHUNK, T_chunk, 128)
        return np.ascontiguousarray(
            a.transpose(0, 4, 1, 2, 3).reshape(ncores, 128, NB * TPB))

    gcounts = gcounts.reshape(NB, NCHUNK)
    return (idx16, to_sbuf(loc_p), to_sbuf(w_p), T_chunk, NB, NPC, last_cnt, CH,
            gcounts)


def build_gnn(nc, N, NPC, NB, T_chunk, dims, ncores, last_cnt, CH, gcounts=None,
              debug=False):
    """Emit the full 5-layer program into nc."""
    L = len(dims) - 1
    TPB = NCHUNK * T_chunk
    SP = T_chunk * 128
    NQ = nc.num_swdge_queues
    qctr = [0]
    dbg_d = {}
    if debug:
        for l in range(L):
            dbg_d[f"dbg_ytab{l}"] = nc.dram_tensor(
                f"dbg_ytab{l}", [N, dims[l + 1]], f32, kind="ExternalOutput")
            if l < L - 1:
                dbg_d[f"dbg_xT{l + 1}"] = nc.dram_tensor(
                    f"dbg_xT{l + 1}", [dims[l + 1], NPC], f32, kind="ExternalOutput")
            dbg_d[f"dbg_agg{l}"] = nc.dram_tensor(
                f"dbg_agg{l}", [NB * 128, dims[l + 1]], f32, kind="ExternalOutput")

    # ---- DRAM I/O ----
    idx_d = nc.dram_tensor("idx16", [NB, 128, NCHUNK * (SP // 16)], i16, kind="ExternalInput")
    dstl_d = nc.dram_tensor("dstl", [128, NB * TPB], f16, kind="ExternalInput")
    wgt_d = nc.dram_tensor("wgt", [128, NB * TPB], f32, kind="ExternalInput")
    xT0_d = nc.dram_tensor("xT0", [dims[0], NPC], f32, kind="ExternalInput")
    x64_d = nc.dram_tensor("x64", [N, PAD], f32, kind="ExternalInput")
    iota_d = nc.dram_tensor("iota", [128, 128], f16, kind="ExternalInput")
    ident_d = nc.dram_tensor("ident", [128, 128], f32, kind="ExternalInput")
    wrel_d = [nc.dram_tensor(f"wrel{l}", [dims[l], dims[l + 1]], f32, kind="ExternalInput") for l in range(L)]
    wroot_d = [nc.dram_tensor(f"wroot{l}", [dims[l], dims[l + 1]], f32, kind="ExternalInput") for l in range(L)]
    brel_d = [nc.dram_tensor(f"brel{l}", [dims[l + 1], 1], f32, kind="ExternalInput") for l in range(L)]
    out_d = nc.dram_tensor("out", [NPC, dims[L]], f32, kind="ExternalOutput")

    groups = [list(range(ncores))]

    with tile.TileContext(nc) as tc:
        with (
            tc.tile_pool(name="const", bufs=1) as cpool,
            tc.tile_pool(name="edge", bufs=1) as epool,
            tc.tile_pool(name="xts", bufs=1) as xpool,
            tc.tile_pool(name="idxp", bufs=4) as ipool,
            tc.tile_pool(name="gath", bufs=4) as gpool,
            tc.tile_pool(name="msg", bufs=3) as mpool,
            tc.tile_pool(name="ohp", bufs=3) as opool,
            tc.tile_pool(name="small", bufs=4) as spool,
            tc.tile_pool(name="psum", bufs=1, space="PSUM") as ppool,
            tc.tile_pool(name="dram", bufs=1, space="DRAM") as dpool,
        ):
            # ---- load constants / edge data ----
            dstl_sb = epool.tile([128, NB * TPB], f16)
            nc.sync.dma_start(dstl_sb[:], dstl_d.ap()[:])
            wgt_sb = epool.tile([128, NB * TPB], f32)
            nc.sync.dma_start(wgt_sb[:], wgt_d.ap()[:])
            iota_sb = cpool.tile([128, 128], f16)
            nc.sync.dma_start(iota_sb[:], iota_d.ap()[:])
            ident_sb = cpool.tile([128, 128], f32)
            nc.sync.dma_start(ident_sb[:], ident_d.ap()[:])
            wrel_sb, wroot_sb, brel_sb = [], [], []
            for l in range(L):
                t1 = cpool.tile([dims[l], dims[l + 1]], f32, name=f"wrel_sb{l}")
                nc.sync.dma_start(t1[:], wrel_d[l].ap()[:])
                wrel_sb.append(t1)
                t2 = cpool.tile([dims[l], dims[l + 1]], f32, name=f"wroot_sb{l}")
                nc.sync.dma_start(t2[:], wroot_d[l].ap()[:])
                wroot_sb.append(t2)
                t3 = cpool.tile([dims[l + 1], 1], f32, name=f"brel_sb{l}")
                nc.sync.dma_start(t3[:], brel_d[l].ap()[:])
                brel_sb.append(t3)

            xT_cur = xpool.tile([dims[0], NPC], f32, name="xT_l0", tag="xT", bufs=2)
            nc.sync.dma_start(xT_cur[:], xT0_d.ap()[:])

            for l in range(L):
                c = dims[l + 1]
                din = dims[l]
                if l == 0:
                    ytab64 = x64_d.ap()
                else:
                    # ---- y = x @ W_rel locally, AllGather, restride to 256B rows
                    ybounce = dpool.tile([NPC, c], f32, name=f"ybounce{l}")
                    for b in range(NB):
                        cnt = 128 if b < NB - 1 else last_cnt
                        yps = ppool.tile([128, c], f32, name=f"yps{l}_{b}", tag="y", bufs=2)
                        nc.tensor.matmul(
                            out=yps[:cnt, :],
                            lhsT=xT_cur[:, b * 128:b * 128 + cnt],
                            rhs=wrel_sb[l][:],
                            start=True, stop=True,
                        )
                        ysb = spool.tile([128, c], f32, name=f"ysb{l}_{b}", tag="ysb")
                        nc.vector.tensor_copy(ysb[:cnt, :], yps[:cnt, :])
                        nc.sync.dma_start(ybounce[b * 128:b * 128 + cnt, :], ysb[:cnt, :])
                    ytabc = dpool.tile([N, c], f32, name=f"ytabc{l}")
                    nc.gpsimd.collective_compute(
                        "AllGather",
                        mybir.AluOpType.bypass,
                        replica_groups=groups,
                        ins=[ybounce[:].opt()],
                        outs=[ytabc[:].opt()],
                    )
                    ytab64_t = dpool.tile([N, PAD], f32, name=f"ytab64_{l}")
                    for ch in range(NCHUNK):
                        r0, r1 = ch * CH, min((ch + 1) * CH, N)
                        nc.sync.dma_start(ytab64_t[r0:r1, :c], ytabc[r0:r1, :])
                    ytab64 = ytab64_t

                # ---- main per-block loop ----
                if l < L - 1:
                    xT_next = xpool.tile([c, NPC], f32, name=f"xT_l{l + 1}", tag="xT", bufs=2)
                else:
                    xT_next = None
                for b in range(NB):
                    cnt = 128 if b < NB - 1 else last_cnt
                    es = slice(b * TPB, (b + 1) * TPB)
                    idx_sb = ipool.tile([128, NCHUNK * (SP // 16)], i16,
                                        name=f"idx{l}_{b}", tag="idx")
                    nc.sync.dma_start(idx_sb[:], idx_d.ap()[b])
                    gth = gpool.tile([128, TPB * PAD], f32, name=f"gth{l}_{b}", tag="gth")
                    if l == 0 and b < 3:
                        # first-touch each of the 3 ring bufs: slots skipped by
                        # short num_idxs_reg must hold finite data (w=0 kills
                        # their contribution, but NaN*0 would not be 0)
                        nc.gpsimd.memset(gth[:], 0)
                    for ch in range(NCHUNK):
                        r0 = ch * CH
                        r1 = min(r0 + CH, N)
                        gmax = SP if gcounts is None else int(gcounts[b][ch])
                        t0 = 0
                        while t0 < T_chunk:
                            tn = min(8, T_chunk - t0)  # dma_gather max 1024 idxs/instr
                            reg = max(0, min(gmax - t0 * 128, tn * 128))
                            if reg == 0:
                                break
                            nc.gpsimd.dma_gather(
                                out_ap=gth[:, (ch * T_chunk + t0) * PAD:(ch * T_chunk + t0 + tn) * PAD]
                                    .rearrange("p (t e) -> p t e", e=PAD),
                                in_ap=ytab64[r0:r1, :],
                                idxs_ap=idx_sb[:, ch * (SP // 16) + t0 * 8:ch * (SP // 16) + (t0 + tn) * 8],
                                num_idxs=tn * 128,
                                num_idxs_reg=reg,
                                elem_size=PAD,
                                queue_num=qctr[0] % NQ,
                            )
                            qctr[0] += 1
                            t0 += tn
                    msg = mpool.tile([128, TPB * c], f16, name=f"msg{l}_{b}", tag="msg")
                    nc.vector.tensor_tensor(
                        out=msg[:].rearrange("p (t c) -> p t c", c=c),
                        in0=gth[:].rearrange("p (t e) -> p t e", e=PAD)[:, :, :c],
                        in1=wgt_sb[:, es].to_broadcast([128, TPB, c]),
                        op=mybir.AluOpType.mult,
                    )
                    oh = opool.tile([128, TPB * 128], f16, name=f"oh{l}_{b}", tag="oh")
                    nc.vector.tensor_tensor(
                        out=oh[:].rearrange("p (t n) -> p t n", n=128),
                        in0=dstl_sb[:, es].to_broadcast([128, TPB, 128]),
                        in1=iota_sb[:, None, :].to_broadcast([128, TPB, 128]),
                        op=mybir.AluOpType.is_equal,
                    )
                    aggps = ppool.tile([128, c], f32, name=f"aggps{l}_{b}", tag="agg", bufs=2)
                    for t in range(TPB):
                        nc.tensor.matmul(
                            out=aggps[:],
                            lhsT=oh[:, t * 128:(t + 1) * 128],
                            rhs=msg[:, t * c:(t + 1) * c],
                            start=(t == 0), stop=(t == TPB - 1),
                        )
                    agg_sb = spool.tile([128, c], f32, name=f"agg_sb{l}_{b}", tag="aggsb")
                    nc.vector.tensor_copy(agg_sb[:], aggps[:])
                    if debug:
                        nc.sync.dma_start(
                            dbg_d[f"dbg_agg{l}"].ap()[b * 128:(b + 1) * 128, :], agg_sb[:])
                    aggT = ppool.tile([c, 128], f32, name=f"aggT{l}_{b}", tag="aggT", bufs=2)
                    nc.tensor.transpose(out=aggT[:], in_=agg_sb[:], identity=ident_sb[:])
                    zps = ppool.tile([c, 128], f32, name=f"zps{l}_{b}", tag="z", bufs=2)
                    nc.tensor.matmul(
                        out=zps[:, :cnt],
                        lhsT=wroot_sb[l][:],
                        rhs=xT_cur[:, b * 128:b * 128 + cnt],
                        start=True, stop=True,
                    )
                    aggT_sb = spool.tile([c, 128], f32, name=f"aggT_sb{l}_{b}", tag="aggTsb")
                    nc.vector.tensor_copy(aggT_sb[:], aggT[:])
                    z_sb = spool.tile([c, 128], f32, name=f"z_sb{l}_{b}", tag="zsb")
                    nc.vector.tensor_add(out=z_sb[:, :cnt], in0=zps[:, :cnt], in1=aggT_sb[:, :cnt])
                    if l < L - 1:
                        nc.scalar.activation(
                            out=xT_next[:, b * 128:b * 128 + cnt],
                            in_=z_sb[:, :cnt],
                            func=mybir.ActivationFunctionType.Relu,
                            bias=brel_sb[l][:],
                        )
                    else:
                        r_sb = spool.tile([c, 128], f32, name=f"r_sb{b}", tag="rsb")
                        nc.scalar.activation(
                            out=r_sb[:, :cnt], in_=z_sb[:, :cnt],
                            func=mybir.ActivationFunctionType.Relu,
                            bias=brel_sb[l][:],
                        )
                        tps = ppool.tile([128, c], f32, name=f"tps{b}", tag="y", bufs=2)
                        nc.tensor.transpose(
                            out=tps[:cnt, :], in_=r_sb[:, :cnt], identity=ident_sb[:c, :c],
                        )
                        zb = spool.tile([128, c], f32, name=f"zb{b}", tag="zb")
                        nc.vector.tensor_copy(zb[:cnt, :], tps[:cnt, :])
                        mx = spool.tile([128, 1], f32, name=f"mx{b}", tag="mx")
                        nc.vector.tensor_tensor(
                            out=mx[:cnt, :], in0=zb[:cnt, 0:1], in1=zb[:cnt, 1:2],
                            op=mybir.AluOpType.max,
                        )
                        zs = spool.tile([128, c], f32, name=f"zs{b}", tag="zs")
                        nc.vector.tensor_tensor(
                            out=zs[:cnt, :], in0=zb[:cnt, :],
                            in1=mx[:cnt, :].to_broadcast([cnt, c]),
                            op=mybir.AluOpType.subtract,
                        )
                        esb = spool.tile([128, c], f32, name=f"esb{b}", tag="esb")
                        nc.scalar.activation(
                            out=esb[:cnt, :], in_=zs[:cnt, :],
                            func=mybir.ActivationFunctionType.Exp,
                        )
                        ssb = spool.tile([128, 1], f32, name=f"ssb{b}", tag="ssb")
                        nc.vector.tensor_add(out=ssb[:cnt, :], in0=esb[:cnt, 0:1], in1=esb[:cnt, 1:2])
                        rcp = spool.tile([128, 1], f32, name=f"rcp{b}", tag="rcp")
                        nc.vector.reciprocal(rcp[:cnt, :], ssb[:cnt, :])
                        osb = spool.tile([128, c], f32, name=f"osb{b}", tag="osb")
                        nc.vector.tensor_tensor(
                            out=osb[:cnt, :], in0=esb[:cnt, :],
                            in1=rcp[:cnt, :].to_broadcast([cnt, c]),
                            op=mybir.AluOpType.mult,
                        )
                        nc.sync.dma_start(out_d.ap()[b * 128:b * 128 + cnt, :], osb[:cnt, :])
                if debug:
                    if l == 0:
                        nc.sync.dma_start(dbg_d[f"dbg_ytab{l}"].ap()[:], x64_d.ap()[:, :dims[1]])
                    else:
                        nc.sync.dma_start(dbg_d[f"dbg_ytab{l}"].ap()[:], ytabc[:])
                    if l < L - 1:
                        nc.sync.dma_start(dbg_d[f"dbg_xT{l + 1}"].ap()[:], xT_next[:])
                xT_cur = xT_next


def make_host_inputs(inputs, N, dims, ncores):
    """From problem inputs, build per-core in_maps for run_bass_kernel_spmd."""
    x = np.asarray(inputs["x"], np.float32)
    (idx16, dstl, wgt, T_chunk, NB, NPC, last_cnt, CH, gcounts) = preprocess(
        inputs["edge_index"], inputs["edge_weight"], N, ncores)
    iota = np.broadcast_to(np.arange(128, dtype=np.float16), (128, 128)).copy()
    ident = np.eye(128, dtype=np.float32)
    # layer-0 gather table: y0 = x @ W_rel0, padded to 256B rows
    x64 = np.zeros((N, PAD), np.float32)
    x64[:, :dims[1]] = x @ np.asarray(inputs["w_rel0"], np.float32)
    L = len(dims) - 1
    common = {"iota": iota, "ident": ident, "x64": x64}
    for l in range(L):
        common[f"wrel{l}"] = np.asarray(inputs[f"w_rel{l}"], np.float32)
        common[f"wroot{l}"] = np.asarray(inputs[f"w_root{l}"], np.float32)
        common[f"brel{l}"] = np.asarray(inputs[f"b_rel{l}"], np.float32).reshape(-1, 1)
    in_maps = []
    for k in range(ncores):
        m = dict(common)
        m["idx16"] = idx16[k]
        m["dstl"] = dstl[k]
        m["wgt"] = wgt[k]
        m["xT0"] = np.ascontiguousarray(x[k * NPC:(k + 1) * NPC].T)
        in_maps.append(m)
    return in_maps, T_chunk, NB, NPC, last_cnt, CH, gcounts


def _install_loud_hook():
    import traceback
    from concourse import bass2jax
    bass2jax.install_neuronx_cc_hook()
    try:
        import libneuronxla
    except ImportError:
        return
    hook = libneuronxla.neuronx_cc
    def loud(*a, **k):
        try:
            return hook(*a, **k)
        except BaseException:
            traceback.print_exc()
            raise
    libneuronxla.neuronx_cc = loud
    bass2jax.install_neuronx_cc_hook = lambda: None


def run_gnn(inputs, N, dims, ncores=8, trace=False, debug=False):
    from concourse.bass_utils import run_bass_kernel_spmd
    _install_loud_hook()
    in_maps, T_chunk, NB, NPC, last_cnt, CH, gcounts = make_host_inputs(
        inputs, N, dims, ncores)
    nc = bacc.Bacc("TRN2", target_bir_lowering=False, debug=False,
                   num_devices=ncores, num_swdge_queues=4)
    build_gnn(nc, N, NPC, NB, T_chunk, dims, ncores, last_cnt, CH,
              gcounts=gcounts, debug=debug)
    nc.compile()
    res = run_bass_kernel_spmd(nc, in_maps, core_ids=list(range(ncores)), trace=trace)
    out = np.concatenate([res.results[k]["out"] for k in range(ncores)], axis=0)
    return out, res


DIMS = [6, 20, 15, 10, 5, 2]
N_NODES = 100000
N_CORES = 8


def kernel(**inputs):
    out, _res = run_gnn(inputs, N_NODES, DIMS, ncores=N_CORES, trace=False)
    return out


def kernel_traced(**inputs):
    """Like kernel() but also returns the BassKernelResults (exec_time_ns etc)."""
    return run_gnn(inputs, N_NODES, DIMS, ncores=N_CORES, trace=True)

